# revision 42
# baseline (speedup 1.0000x reference)
"""Paged-attention decode (GQA) on 8 Trainium2 NeuronCores.

Strategy (data-parallel over 128-token tiles):
  - Host gathers each sequence's valid KV blocks (via block_table/seq_lens)
    into packed 128-token tiles: K transposed to [D=128, L] per KV head,
    V natural [L, D=128] per KV head, plus a validity column (for the
    softmax denominator matmul).
  - Tiles are distributed evenly across the 8 cores (each tile = same cost).
  - Precision: the kernel is HBM-bandwidth bound, so KV bytes are
    everything. Sequences with L >= 512 tokens ship K/V in fp8 (e3m4:
    4 mantissa bits); shorter sequences (whose softmax averages over
    fewer tokens and so amplifies quantization noise the most) stay in
    bf16. q and p (the exp'd scores) stay bf16 -- the tensor engine
    accepts mixed-dtype operands. Accumulation is fp32 PSUM; the final
    combine runs on host in float64. End-to-end rel err ~1.1e-2
    (gate 2e-2) -- validated offline against the fp64 reference; the
    bf16-only variant of this pipeline reproduced its offline sim
    error to 4 digits on hardware.
  - No masking is needed: padded tokens have K=V=0 so scores=0, p=1,
    but V=0 keeps them out of the numerator and the valid column keeps
    them out of the denominator.
  - Device, per tile: 8 QK matmuls (K_h stationary, q streams) ->
    scores [128L, 32hg] in PSUM; one ScalarE exp writes p (bf16) into
    per-head 32-col zero-padded weight windows; 8 col-tiled PV matmuls
    (p_h stationary in PE column group h%4, [V_h | valid] streams) ->
    transposed accumulators accT[g, d] plus the softmax denominator as
    column 128, heads pair-concurrent across the PE column groups so
    the weight-load port only ever carries K. One DVE cast-copy stages
    each tile's [128, 258] PSUM; partition-remap stores ship the four
    4-row stripes to the compact [16, nt*258] output. KV streams in
    ~1 MiB DMA chunks; finished outputs stream back incrementally.
  - Host sums per-tile partial numerators/denominators per sequence and
    normalizes (the standard distributed-softmax combine).
"""

import math

import numpy as np

# Problem constants (hardcoded per task contract).
NUM_SEQS = 32
NUM_HEADS = 32
NUM_KV_HEADS = 8
GQA = NUM_HEADS // NUM_KV_HEADS  # 4
HEAD_SIZE = 128
BLOCK_SIZE = 16
MAX_BLOCKS_PER_SEQ = 128
MAX_SEQ_LEN = MAX_BLOCKS_PER_SEQ * BLOCK_SIZE
SCALE = 1.0 / math.sqrt(HEAD_SIZE)
N_CORES = 8
TILE_L = 128          # tokens per device tile
FP8_MIN_L = 512       # sequences at least this long ship KV in fp8
HG = NUM_HEADS        # 32 (kv_head-major query head order)
HB = NUM_KV_HEADS * HEAD_SIZE      # 1024 cols for the K plane
VB = HEAD_SIZE + 1                 # 129: V_h block + its valid column
KV_COLS = HB + NUM_KV_HEADS * VB   # 2056: K | 8 x (V_h | valid)
OB = 2 * VB                        # 258 out cols/tile: accT_A|den, accT_B|den

_PROGRAM_CACHE = {}
LAST_RUN = None  # BassKernelResults of the most recent run (for test harness)


def _build_program(n16: int, n8: int):
    """Build the SPMD Bass/Tile program: per core, n16 bf16 KV tiles
    followed by n8 fp8(e3m4) KV tiles."""
    import concourse.bacc as bacc
    import concourse.mybir as mybir
    import concourse.tile as tile

    f32 = mybir.dt.float32
    bf16 = mybir.dt.bfloat16
    fp8 = mybir.dt.float8e3
    nt = n16 + n8
    nc = bacc.Bacc("TRN2", target_bir_lowering=False, debug=False,
                   num_devices=N_CORES)

    kv16_d = nc.dram_tensor("kv16", [128, max(n16, 1) * KV_COLS], bf16,
                            kind="ExternalInput")
    kv8_d = nc.dram_tensor("kv8", [128, max(n8, 1) * KV_COLS], fp8,
                           kind="ExternalInput")
    q_d = nc.dram_tensor("q", [128, nt * HG], bf16, kind="ExternalInput")
    # bf16 staging/output: halves output DMA; validated offline to leave
    # the end-to-end rel err at ~1.09e-2 (the combine runs in float64).
    # Transposed-compact layout: row 4j+r of the A half = (kv head j,
    # query group r); the B half = kv heads 4-7.
    out_d = nc.dram_tensor("out", [16, nt * OB], bf16,
                           kind="ExternalOutput")

    with tile.TileContext(nc) as tc:
        with (
            tc.tile_pool(name="const", bufs=1) as const_pool,
            tc.tile_pool(name="kv16p", bufs=1) as kv16_pool,
            tc.tile_pool(name="kv8p", bufs=7) as kv8_pool,
            tc.tile_pool(name="pp", bufs=1) as p_pool,
            tc.tile_pool(name="acc_sb", bufs=1) as stage_pool,
            tc.tile_pool(name="ps_s", bufs=4, space="PSUM") as ps_scores,
            tc.tile_pool(name="ps_o", bufs=4, space="PSUM") as ps_acc,
        ):
            # q first on the same (sync/HWDGE) queue as KV so it does not
            # compete with the KV stream for HBM bandwidth mid-kernel.
            qt = const_pool.tile([128, nt * HG], bf16)
            nc.sync.dma_start(out=qt[:], in_=q_d.ap())
            out_stage = stage_pool.tile([128, nt * OB], bf16)
            nc.vector.memset(out_stage[:], 0.0)
            # p buffers: each head's 4 exp'd-score columns sit at the top
            # of their own 32-col window so the [128, 32] window is a
            # ready-made PE weight whose cols 4-31 are zero (the zero
            # cols only produce never-read output rows; the memset runs
            # once, the pad columns are never written again)
            NPB = 4
            p256 = p_pool.tile([128, NPB * 256], bf16)
            nc.vector.memset(p256[:], 0.0)

            # DMA chunk schedule. bf16 tiles (if any) come first in one
            # chunk; fp8 tiles stream in 4-tile (~1 MiB) chunks, tapering
            # to 2/1-tile chunks at the end. Prefetch depth is bounded
            # (~20 tiles in flight): racing the whole stream in at full
            # rate was measured to coincide with a chip P0 power-state
            # downclock (PE 2.4 -> 2.0 GHz), costing more tensor time
            # than the early DMA finish saves.
            sizes = []
            r = n8
            while r > 5:
                sizes.append(4)
                r -= 4
            sizes += {5: [2, 2, 1], 4: [2, 1, 1], 3: [2, 1],
                      2: [1, 1], 1: [1], 0: []}[r]
            starts = [sum(sizes[:i]) for i in range(len(sizes))]

            chunk_tiles = {}
            if n16:
                # per-tile DMAs so the first matmuls start as soon as the
                # first ~0.5 MB lands instead of after the whole block
                ct = kv16_pool.tile([128, n16 * KV_COLS], bf16)
                for i in range(n16):
                    nc.sync.dma_start(
                        out=ct[:, i * KV_COLS:(i + 1) * KV_COLS],
                        in_=kv16_d.ap()[:, i * KV_COLS:(i + 1) * KV_COLS])
                    chunk_tiles[i] = ct[:, i * KV_COLS:(i + 1) * KV_COLS]
            big = max(sizes) if sizes else 1
            for ci, (sz, st) in enumerate(zip(sizes, starts)):
                ct = kv8_pool.tile([128, big * KV_COLS], fp8)
                c0 = st * KV_COLS
                if ci >= len(sizes) - 2 and sz == 1:
                    # split the last tiles' DMA into K-plane then V-plane
                    # so their QK matmuls overlap the V transfer
                    # (shortens the end-of-kernel serial drain)
                    nc.sync.dma_start(
                        out=ct[:, :HB],
                        in_=kv8_d.ap()[:, c0:c0 + HB])
                    nc.sync.dma_start(
                        out=ct[:, HB:KV_COLS],
                        in_=kv8_d.ap()[:, c0 + HB:c0 + KV_COLS])
                else:
                    nc.sync.dma_start(
                        out=ct[:, :sz * KV_COLS],
                        in_=kv8_d.ap()[:, c0:c0 + sz * KV_COLS])
                for i in range(sz):
                    chunk_tiles[n16 + st + i] = ct[:, i * KV_COLS:
                                                   (i + 1) * KV_COLS]

            OUT_CHUNK = 12  # tiles per incremental output store
            out_done = 0   # tiles whose output has been stored

            for t in range(nt):
                kvt = chunk_tiles[t]

                # scores[l, h*4+g] = sum_d K[l,d] * q_scaled[h,g,d]
                scores = ps_scores.tile([128, HG], f32)
                qb = t * HG
                for h in range(NUM_KV_HEADS):
                    nc.tensor.matmul(
                        scores[:, h * GQA:(h + 1) * GQA],
                        kvt[:, h * HEAD_SIZE:(h + 1) * HEAD_SIZE],
                        qt[:, qb + h * GQA:qb + (h + 1) * GQA],
                        start=True, stop=True)

                # p = exp(scores) in bf16, written into the head-strided
                # weight windows of this tile's p buffer
                pb = p256[:, (t % NPB) * 256:(t % NPB) * 256 + 256]
                nc.scalar.activation(
                    pb.rearrange("p (h c) -> p h c", c=32)[:, :, 0:GQA],
                    scores[:].rearrange("p (h g) -> p h g", g=GQA),
                    mybir.ActivationFunctionType.Exp)

                # PV transposed: accT[g, d] = sum_l p[l, h*4+g] * V[l,h,d],
                # with the per-head valid column streamed as a 129th rhs
                # column so accT[g, 128] = the (h,g) softmax denominator.
                # p is the stationary operand and V streams, so the PE
                # weight-load port only ever carries K; the four column
                # groups of the PE array run head-pairs concurrently.
                # acc cols 0:129 = heads 0-3 (A half), 129:258 = heads 4-7
                # (B half); 258 fp32 cols fit one PSUM bank
                acc = ps_acc.tile([128, OB], f32)
                for h in range(NUM_KV_HEADS):
                    j = h % 4
                    cb = 0 if h < NUM_KV_HEADS // 2 else VB
                    nc.tensor.matmul(
                        acc[32 * j:32 * j + 32, cb:cb + VB],
                        pb[:, 32 * h:32 * h + 32],
                        kvt[:, HB + h * VB:HB + (h + 1) * VB],
                        start=True, stop=True,
                        tile_position=(0, 32 * j))

                base = t * OB
                nc.vector.tensor_copy(out_stage[:, base:base + OB], acc[:])

                # stream finished output chunks while KV is still loading.
                # Only the 4-row stripe at the top of each 32-partition
                # group holds data; one plain 2D store per group remaps
                # partitions {32j..32j+3} to output rows {4j..4j+3}.
                emit = (t == nt - 1 or t == nt - 4 or
                        (t % OUT_CHUNK == OUT_CHUNK - 1 and t < nt - 4))
                if emit:
                    c0 = out_done * OB
                    c1 = (t + 1) * OB
                    out_done = t + 1
                    for j in range(4):
                        # split the 4 group-stores across both HWDGE
                        # issue engines so the final emit's instruction
                        # issue time (~640ns each) halves; by this point
                        # the sync engine has issued every KV chunk DMA,
                        # so stalling it on the copy semaphore is safe
                        eng = nc.scalar if j % 2 == 0 else nc.sync
                        eng.dma_start(
                            out=out_d.ap()[4 * j:4 * j + 4, c0:c1],
                            in_=out_stage[32 * j:32 * j + 4, c0:c1])

    nc.compile()
    return nc


def _prepare(query, key_cache, value_cache, block_table, seq_lens):
    """Shard FULL inputs into per-core SPMD input maps. Returns
    (in_maps, assign, n16, n8) where assign[c] = [(slot, seq), ...]."""
    import ml_dtypes
    bf16 = ml_dtypes.bfloat16
    fp8 = ml_dtypes.float8_e3m4
    S = query.shape[0]
    lens = [int(x) for x in seq_lens]

    # ---- host-side shard: per-dtype global tile lists (seq, offset, n)
    tiles16, tiles8 = [], []
    for s in range(S):
        L = lens[s]
        dst = tiles8 if L >= FP8_MIN_L else tiles16
        for t0 in range(0, L, TILE_L):
            dst.append((s, t0, min(TILE_L, L - t0)))
    n16 = (len(tiles16) + N_CORES - 1) // N_CORES
    n8 = (len(tiles8) + N_CORES - 1) // N_CORES
    nt = n16 + n8

    # q^T, kv_head-major, pre-scaled: [d, s*32 + h*4 + g]
    q_hg = query.reshape(S, HG, HEAD_SIZE) * np.float32(SCALE)  # [s, hg, d]
    qT_all = np.ascontiguousarray(
        q_hg.reshape(S * HG, HEAD_SIZE).T).astype(bf16)

    # Gather each sequence's valid KV via block_table (the paged layout),
    # transpose K to [d, h, l].
    kseq, vseq = {}, {}
    for s in range(S):
        L = lens[s]
        nblk = (L + BLOCK_SIZE - 1) // BLOCK_SIZE
        blocks = block_table[s, :nblk].astype(np.int64)
        k = key_cache[blocks].reshape(nblk * BLOCK_SIZE, NUM_KV_HEADS,
                                      HEAD_SIZE)[:L]
        v = value_cache[blocks].reshape(nblk * BLOCK_SIZE, NUM_KV_HEADS,
                                        HEAD_SIZE)[:L]
        dt = fp8 if L >= FP8_MIN_L else bf16
        kseq[s] = np.ascontiguousarray(k.transpose(2, 1, 0)).astype(dt)
        vseq[s] = v.reshape(L, NUM_KV_HEADS * HEAD_SIZE).astype(dt)

    in_maps = []
    assign = []  # per core: list of (slot, seq)
    for c in range(N_CORES):
        kv16 = np.zeros((max(n16, 1), 128, KV_COLS), dtype=bf16)
        kv8 = np.zeros((max(n8, 1), 128, KV_COLS), dtype=fp8)
        qc = np.zeros((128, nt * HG), dtype=bf16)
        slots = []

        def fill(kv_all, tiles, cnt, slot0):
            for i in range(cnt):
                gi = c * cnt + i
                if gi >= len(tiles):
                    continue
                s, t0, n = tiles[gi]
                kv = kv_all[i]
                kv[:, :HB].reshape(128, NUM_KV_HEADS, HEAD_SIZE)[
                    :, :, :n] = kseq[s][:, :, t0:t0 + n]
                vb = kv[:, HB:].reshape(128, NUM_KV_HEADS, VB)
                vb[:n, :, :HEAD_SIZE] = vseq[s][t0:t0 + n].reshape(
                    n, NUM_KV_HEADS, HEAD_SIZE)
                vb[:n, :, HEAD_SIZE] = kv.dtype.type(1.0)
                slot = slot0 + i
                qb = slot * HG
                qc[:, qb:qb + HG] = qT_all[:, s * HG:(s + 1) * HG]
                slots.append((slot, s))

        fill(kv16, tiles16, n16, 0)
        fill(kv8, tiles8, n8, n16)
        in_maps.append({
            "kv16": np.ascontiguousarray(
                kv16.transpose(1, 0, 2).reshape(128, -1)),
            "kv8": np.ascontiguousarray(
                kv8.transpose(1, 0, 2).reshape(128, -1)),
            "q": qc,
        })
        assign.append(slots)
    return in_maps, assign, n16, n8


def _combine(results, assign, S, nt):
    """Sum per-tile partial numerators/denominators per sequence, normalize.
    Returns None if the results look corrupted (e.g. a core transiently
    returned zeros -> denominator <= 0), so the caller can retry."""
    num = np.zeros((S, HG, HEAD_SIZE), dtype=np.float64)
    den = np.zeros((S, HG), dtype=np.float64)
    for c in range(N_CORES):
        o = results[c]["out"].astype(np.float32)  # [16, nt*258] (bf16)
        for slot, s in assign[c]:
            blk = o[:, slot * OB:(slot + 1) * OB]
            if not np.isfinite(blk).all():
                return None
            a, b = blk[:, :VB], blk[:, VB:]
            num[s][:16] += a[:, :HEAD_SIZE]
            num[s][16:] += b[:, :HEAD_SIZE]
            den[s][:16] += a[:, HEAD_SIZE]
            den[s][16:] += b[:, HEAD_SIZE]
    if not (den > 0).all():
        return None
    out = (num / den[:, :, None]).astype(np.float32)
    if not np.isfinite(out).all():
        return None
    return out.reshape(S, NUM_HEADS * HEAD_SIZE)


def kernel(query, key_cache, value_cache, block_table, seq_lens):
    query = np.ascontiguousarray(np.asarray(query, dtype=np.float32))
    key_cache = np.asarray(key_cache, dtype=np.float32)
    value_cache = np.asarray(value_cache, dtype=np.float32)
    block_table = np.asarray(block_table, dtype=np.int32)
    seq_lens = np.asarray(seq_lens, dtype=np.int32)

    in_maps, assign, n16, n8 = _prepare(query, key_cache, value_cache,
                                        block_table, seq_lens)

    # bass_utils imports antenv.axon_hooks when tracing is requested; the
    # image's antenv lacks that module, so synthesize a shim defensively.
    try:
        import antenv.axon_hooks  # noqa: F401
    except ImportError:
        try:
            import sys
            import types

            import antenv
            mod = types.ModuleType("antenv.axon_hooks")
            mod._hook = None
            mod.set_axon_ntff_profile_hook = \
                lambda h: setattr(mod, "_hook", h)
            mod.get_axon_ntff_profile_hook = lambda: mod._hook
            sys.modules["antenv.axon_hooks"] = mod
            antenv.axon_hooks = mod
            from trn_agent_boot.trn_boot import _ntff_profile_via_ctypes
            mod._hook = _ntff_profile_via_ctypes("/opt/axon/libaxon_pjrt.so")
        except Exception:  # noqa: BLE001 - tracing is optional
            pass

    from concourse.bass_utils import run_bass_kernel_spmd

    key = (n16, n8)
    if key not in _PROGRAM_CACHE:
        _PROGRAM_CACHE[key] = _build_program(n16, n8)
    nc = _PROGRAM_CACHE[key]

    global LAST_RUN
    out = None
    for attempt in range(3):
        br = run_bass_kernel_spmd(nc, in_maps, list(range(N_CORES)))
        LAST_RUN = br
        out = _combine(br.results, assign, query.shape[0], n16 + n8)
        if out is not None:
            break
        # transient device glitch (a core returned zeros/NaNs) -> retry
    assert out is not None, "device returned corrupted results 3x"
    return out
